# revision 8
# baseline (speedup 1.0000x reference)
"""Trainium2 Bass kernel for top-2 MoE routing (nn_JaxMoE_26431228740246).

Strategy: expert parallel across 8 NeuronCores (1 expert per core) with
SPARSE dispatch.  The reference computes every expert densely over all 2048
tokens, but only the top-2 experts per token carry nonzero combine weight, so
each core only needs its expert's assigned tokens (~512 avg, 551 max here).

Host side (the dispatch/combine layer of expert parallelism): router
softmax + top-2 + renormalize in fp32 numpy, gather each expert's tokens
into fixed-capacity slabs already in SBUF partition layout (capacity C
rounded up from the actual max load), scatter-add the weighted expert
outputs back to [T, D].  All weight tensors are pre-permuted on host into
per-partition-contiguous DMA blocks; the first gate/up block is split small
(128 f-cols) so the PE can start after ~0.8MB of DMA.

Device side (per core): plain SwiGLU MLP over C tokens in bf16 —
h = silu(x@Wg) * (x@Wu); out = h @ Wd — weights streamed over both HWDGE
queues (SP + Activation), fp32 PSUM accumulation, output [128, DT, C] fp32.
A short chain of dummy matmuls on zeroed scratch warms the PE p-state ramp
while the first DMAs land.  No router, no transpose, no collectives.

Shapes (hardcoded): T=2048, D=1024, F=4096, E=8, K=2.
"""

import os
import sys

import numpy as np
import ml_dtypes


def _ensure_path():
    for p in (
        "/root/.axon_site",
        "/root/.axon_site/_ro/trn_rl_repo",
        "/root/.axon_site/_ro/pypackages",
        "/opt/trn_rl_repo",
    ):
        if os.path.isdir(p) and p not in sys.path:
            sys.path.append(p)


_ensure_path()

T, D, F, E = 2048, 1024, 4096, 8
DT = D // 128       # 8 d-tiles
FTILES = F // 128   # 32 f-tiles
DC = 256            # d columns per down-proj weight DMA chunk
NDC = D // DC       # 4 chunks

# gate/up weight stream blocks (f-offset, width): first block small so the
# first MM group starts early
W_BLOCKS = [(0, 128), (128, 384)] + [(f0, 512) for f0 in range(512, F, 512)]

_CACHE = {}


def _chunks(C):
    out, c0 = [], 0
    while c0 < C:
        cw = min(512, C - c0)
        out.append((c0, cw))
        c0 += cw
    return out


def _build(C):
    import concourse.tile as tile
    from concourse import bacc, mybir

    fp32 = mybir.dt.float32
    bf16 = mybir.dt.bfloat16
    Act = mybir.ActivationFunctionType

    chunks = _chunks(C)

    nc = bacc.Bacc("TRN2", target_bir_lowering=False, debug=False, num_devices=E)

    # every input tensor is one contiguous DMA block (host pre-permuted)
    xls = {
        c0: nc.dram_tensor(f"xl{c0}", [128, DT, cw], bf16, kind="ExternalInput").ap()
        for c0, cw in chunks
    }
    wgs, wus = [], []
    for j, (f0, w) in enumerate(W_BLOCKS):
        wgs.append(
            nc.dram_tensor(f"wg{j}", [128, DT, w], bf16, kind="ExternalInput").ap()
        )
        wus.append(
            nc.dram_tensor(f"wu{j}", [128, DT, w], bf16, kind="ExternalInput").ap()
        )
    wdl = nc.dram_tensor("wdl", [128, NDC, FTILES, DC], bf16, kind="ExternalInput").ap()
    out = nc.dram_tensor("out", [128, DT, C], fp32, kind="ExternalOutput").ap()

    from contextlib import ExitStack

    with tile.TileContext(nc) as tc, ExitStack() as ctx:
        pconst = ctx.enter_context(tc.tile_pool(name="const", bufs=1))
        ph = ctx.enter_context(tc.tile_pool(name="h", bufs=1))
        pwgu = ctx.enter_context(tc.tile_pool(name="wgu", bufs=2))
        pwd = ctx.enter_context(tc.tile_pool(name="wd", bufs=2))
        posb = ctx.enter_context(tc.tile_pool(name="osb", bufs=2))
        ptmp = ctx.enter_context(tc.tile_pool(name="tmp", bufs=2))
        pwarm = ctx.enter_context(tc.tile_pool(name="warm", bufs=1, space="PSUM"))
        pmm = {
            cw: ctx.enter_context(
                tc.tile_pool(name=f"mm{cw}", bufs=4 if cw >= 512 else 3, space="PSUM")
            )
            for cw in sorted({cw for _, cw in chunks})
        }

        # x chunk-0 slab first on both queues: the first MM group needs it
        c0_, cw_ = chunks[0]
        xs0 = pconst.tile([128, DT, cw_], bf16, tag="xsb0")
        nc.sync.dma_start(xs0[:, : DT // 2, :], xls[c0_][:, : DT // 2, :])
        nc.scalar.dma_start(xs0[:, DT // 2 :, :], xls[c0_][:, DT // 2 :, :])
        xsb = {c0_: xs0}

        # PE warm-up on zeroed scratch while the first DMAs land (TRN2 PE
        # p-state ramp needs ~3us of sustained activity to hit 2.4 GHz)
        wz = pconst.tile([128, 128], bf16, tag="wz")
        nc.vector.memzero(wz[:])
        mz = pconst.tile([128, 256], bf16, tag="mz")
        nc.vector.memzero(mz[:])
        pwv = pwarm.tile([128, 256], fp32, tag="warm")
        NWARM = 16
        for i in range(NWARM):
            nc.tensor.matmul(
                pwv[:], wz[:], mz[:], start=(i == 0), stop=(i == NWARM - 1)
            )

        h = ph.tile([128, FTILES, C], bf16, tag="h")

        # ---- gate/up -> h ----
        for j, (f0, w) in enumerate(W_BLOCKS):
            wg_t = pwgu.tile([128, DT, w], bf16, tag=f"wg{w}")
            nc.sync.dma_start(wg_t[:], wgs[j][:])
            wu_t = pwgu.tile([128, DT, w], bf16, tag=f"wu{w}")
            nc.scalar.dma_start(wu_t[:], wus[j][:])
            if j == 0:
                # remaining x chunks: needed from the second MM group on
                for c0, cw in chunks[1:]:
                    t = pconst.tile([128, DT, cw], bf16, tag=f"xsb{c0}")
                    nc.sync.dma_start(t[:, : DT // 2, :], xls[c0][:, : DT // 2, :])
                    nc.scalar.dma_start(t[:, DT // 2 :, :], xls[c0][:, DT // 2 :, :])
                    xsb[c0] = t
            for fi in range(w // 128):
                k = (f0 // 128) + fi
                for c0, cw in chunks:
                    xs = xsb[c0]
                    pg = pmm[cw].tile([128, cw], fp32, tag=f"mm{cw}")
                    for do in range(DT):
                        nc.tensor.matmul(
                            pg[:],
                            wg_t[:, do, fi * 128 : (fi + 1) * 128],
                            xs[:, do, :],
                            start=(do == 0),
                            stop=(do == DT - 1),
                        )
                    tmp = ptmp.tile([128, cw], fp32, tag=f"tmp{cw}")
                    nc.scalar.activation(tmp[:], pg[:], Act.Silu)
                    pu = pmm[cw].tile([128, cw], fp32, tag=f"mm{cw}")
                    for do in range(DT):
                        nc.tensor.matmul(
                            pu[:],
                            wu_t[:, do, fi * 128 : (fi + 1) * 128],
                            xs[:, do, :],
                            start=(do == 0),
                            stop=(do == DT - 1),
                        )
                    nc.vector.tensor_mul(h[:, k, c0 : c0 + cw], tmp[:], pu[:])

        # ---- down-projection ----
        for dp in range(NDC):
            wd_t = pwd.tile([128, FTILES, DC], bf16, tag="wd")
            dma_eng = nc.sync if dp % 2 == 0 else nc.scalar
            dma_eng.dma_start(wd_t[:], wdl[:, dp])
            for di in range(DC // 128):
                dd = dp * (DC // 128) + di
                for c0, cw in chunks:
                    po = pmm[cw].tile([128, cw], fp32, tag=f"mm{cw}")
                    for k in range(FTILES):
                        nc.tensor.matmul(
                            po[:],
                            wd_t[:, k, di * 128 : (di + 1) * 128],
                            h[:, k, c0 : c0 + cw],
                            start=(k == 0),
                            stop=(k == FTILES - 1),
                        )
                    osb = posb.tile([128, cw], fp32, tag=f"osb{cw}")
                    nc.vector.tensor_copy(osb[:], po[:])
                    nc.scalar.dma_start(out[:, dd, c0 : c0 + cw], osb[:])

    nc.compile()
    return nc


def _get_nc(C):
    key = ("nc", C)
    if key not in _CACHE:
        _CACHE[key] = _build(C)
    return _CACHE[key]


_BF = ml_dtypes.bfloat16


def _wblock(w_DF, f0, w):
    # [D, f0:f0+w] -> [128, DT, w] partition layout, contiguous
    return np.ascontiguousarray(
        w_DF[:, f0 : f0 + w].reshape(DT, 128, w).transpose(1, 0, 2)
    ).astype(_BF)


def kernel(
    x_TD, w_router_DE, kernel_gating_EDF, kernel_up_proj_EDF, kernel_down_proj_EFD
):
    from concourse.bass_utils import run_bass_kernel_spmd

    x = np.ascontiguousarray(np.asarray(x_TD, dtype=np.float32))
    wr = np.ascontiguousarray(np.asarray(w_router_DE, dtype=np.float32))
    g = np.asarray(kernel_gating_EDF, dtype=np.float32)
    u = np.asarray(kernel_up_proj_EDF, dtype=np.float32)
    d = np.asarray(kernel_down_proj_EFD, dtype=np.float32)

    # ---- router (fp32, exact top-2 + renormalize) ----
    logits = x @ wr
    p = np.exp(logits - logits.max(axis=-1, keepdims=True))
    p /= p.sum(axis=-1, keepdims=True)
    rows = np.arange(T)
    i1 = p.argmax(axis=-1)
    p2 = p.copy()
    p2[rows, i1] = -1.0
    i2 = p2.argmax(axis=-1)
    v1, v2 = p[rows, i1], p[rows, i2]
    s = v1 + v2
    w1, w2 = v1 / s, v2 / s

    idxs, wts = [], []
    for e in range(E):
        m1 = i1 == e
        sel = m1 | (i2 == e)
        idx = np.nonzero(sel)[0]
        idxs.append(idx)
        wts.append(np.where(m1, w1, w2)[idx].astype(np.float32))

    L = max(len(ix) for ix in idxs)
    C = max(512, -(-L // 8) * 8)  # capacity from the actual loads
    chunks = _chunks(C)
    nc = _get_nc(C)

    in_maps = []
    for e in range(E):
        xT = x[idxs[e]].T  # [D, L]
        L_e = xT.shape[1]
        m = {}
        for c0, cw in chunks:
            xe = np.zeros((128, DT, cw), dtype=_BF)
            n = max(0, min(cw, L_e - c0))
            if n:
                xe[:, :, :n] = (
                    xT[:, c0 : c0 + n].reshape(DT, 128, n).transpose(1, 0, 2)
                )
            m[f"xl{c0}"] = xe
        for j, (f0, w) in enumerate(W_BLOCKS):
            m[f"wg{j}"] = _wblock(g[e], f0, w)
            m[f"wu{j}"] = _wblock(u[e], f0, w)
        m["wdl"] = d[e].reshape(FTILES, 128, NDC, DC).transpose(1, 2, 0, 3).astype(_BF)
        in_maps.append(m)

    trace = bool(os.environ.get("BASS_PROF"))
    try:
        res = run_bass_kernel_spmd(nc, in_maps, list(range(E)), trace=trace)
    except Exception:
        if not trace:
            raise
        res = run_bass_kernel_spmd(nc, in_maps, list(range(E)), trace=False)
    _CACHE["last_result"] = res

    out = np.zeros((T, D), dtype=np.float32)
    for e in range(E):
        ye = np.asarray(res.results[e]["out"], dtype=np.float32)  # [128, DT, C]
        ye = ye.transpose(1, 0, 2).reshape(D, C)
        out[idxs[e]] += wts[e][:, None] * ye[:, : len(idxs[e])].T
    return out
